# revision 21
# baseline (speedup 1.0000x reference)
"""Trainium2 Bass kernel for 4-head spatial attention score softmax.

Reference computation:
    qk = einsum('bcxy,oc->boxy', fmap[1,256,64,64], W_qk[1024,256])
    q, k = split(qk, 2, axis=1)             # each [1, 512, 64, 64]
    q = q reshaped to heads, scaled by 128^-0.5
    sim[b,h,xy,uv] = q . k  (contraction over dim_head=128)
    out = softmax(sim, axis=-1)             # [1, 4, 4096, 4096] f32

Sharding: 8 cores = 4 heads x 2 query-halves. Each core projects q for its
2048 query columns + k for all 4096 columns (both via PE matmuls over the
channel dim), computes scores with f32r (FP22) matmuls, softmax
(exp on ScalarE with accumulated row sums, normalize on VectorE), and
streams its [2048, 4096] f32 output slab to HBM.
"""

import numpy as np

import concourse.bacc as bacc
import concourse.mybir as mybir
import concourse.tile as tile
from concourse import bass_utils

HEADS = 4
DIM_HEAD = 128
C = 256          # input channels
XY = 4096        # 64*64 spatial positions
QCHUNK = 2048    # query positions per core
N_CORES = 8
SCALE = DIM_HEAD ** -0.5

F32 = mybir.dt.float32
F32R = mybir.dt.float32r
BF16 = mybir.dt.bfloat16

import concourse.bass as bass

# dtype of the q/k operands of the big score matmuls. 16-bit halves the PE
# streaming cost vs f32r (4-byte moving operand streams at ~2 cyc/elem) and
# enables fast weight load. fp16 over bf16: q/k are O(1), so the e5m10
# mantissa (exact inside the PE's FP22) cuts quantization error ~8x.
# NOTE: both operands MUST share one dtype - mixing fp16/bf16 in a single
# matmul hard-crashes the device (NRT_EXEC_UNIT_UNRECOVERABLE).
QK_DT = mybir.dt.float16


def _emit(tc, fmap_k, wqkt, out):
    nc = tc.nc

    with tc.tile_pool(name="consts", bufs=1) as consts:
        # Weights transposed on host: [c, d] with c split into 2 partition chunks.
        # wqkt = [wq.T | wk.T] concatenated: one DMA instead of two.
        w_sb = consts.tile([128, 2, 2 * DIM_HEAD], F32R)
        # fmap [256, n] -> [128p, 2, n]
        fk_sb = consts.tile([128, 2, XY], F32R)
        warm_sb = consts.tile([128, 512], QK_DT)
        fk_src = fmap_k.rearrange("(a p) n -> p a n", p=128)
        nc.sync.dma_start(out=w_sb, in_=wqkt.rearrange("(a p) d -> p a d", p=128))
        # fmap_k in column chunks so the k projection overlaps the load
        KCH = 1024
        for c in range(XY // KCH):
            nc.sync.dma_start(out=fk_sb[:, :, c * KCH:(c + 1) * KCH],
                              in_=fk_src[:, :, c * KCH:(c + 1) * KCH])

        q_sb = consts.tile([128, QCHUNK], QK_DT)  # [d, x] for this core's queries
        k_sb = consts.tile([128, XY], QK_DT)      # [d, uv]

        nc.vector.memset(warm_sb, 0.0)

        # One PSUM pool + tag for warmup, projections, and scores: a second
        # pool would overlap the first's banks and pick up a release
        # dependency on the *last* projection, stalling the first score
        # matmuls behind work they don't need.
        with tc.tile_pool(name="ps", bufs=2, space="PSUM") as ps_pool, \
             tc.tile_pool(name="soft", bufs=6) as soft_pool, \
             tc.tile_pool(name="small", bufs=4) as small_pool:
            # PE warmup: dummy matmuls with no load deps keep TensorE busy
            # through the input-DMA window, so the HAM clock gate is at
            # 2.4 GHz by the time real matmuls arrive (cold PE at startup
            # was the serialization bottleneck).
            warm_ps = ps_pool.tile([128, 2048], F32, tag="ps")
            for i in range(8):
                nc.tensor.matmul(warm_ps[:, 0:512], lhsT=warm_sb[:, 0:128],
                                 rhs=warm_sb, start=True, stop=True)

            # ---- k projection: out[d, n] = sum_c W^T[c, d] * fmap[c, n] ----
            def emit_kproj(g):
                ps_k = ps_pool.tile([128, 2048], F32, tag="ps",
                                    name=f"ps_k{g}")
                for c2 in range(2):
                    c = g * 2 + c2
                    for j in range(KCH // 512):
                        osl = slice(c2 * KCH + j * 512, c2 * KCH + (j + 1) * 512)
                        ksl = slice(c * KCH + j * 512, c * KCH + (j + 1) * 512)
                        nc.tensor.matmul(ps_k[:, osl],
                                         lhsT=w_sb[:, 0, DIM_HEAD:2 * DIM_HEAD],
                                         rhs=fk_sb[:, 0, ksl],
                                         start=True, stop=False)
                        nc.tensor.matmul(ps_k[:, osl],
                                         lhsT=w_sb[:, 1, DIM_HEAD:2 * DIM_HEAD],
                                         rhs=fk_sb[:, 1, ksl],
                                         start=False, stop=True)
                    nc.vector.tensor_copy(
                        k_sb[:, c * KCH:(c + 1) * KCH],
                        ps_k[:, c2 * KCH:(c2 + 1) * KCH])

            # ---- q projection from fk_sb (no separate fmap_q transfer):
            # this core's query columns are fmap columns
            # [qhalf*2048, qhalf*2048+2048), selected with a dynamic offset
            # from the partition id (core 2h+qhalf handles head h, half qhalf).
            qoff = (nc.tensor.partition_id() % 2) * QCHUNK

            def emit_warm(n, tag_i=[0]):
                # keep the HAM clock gate warm between projection chunks;
                # fresh tile per burst so no long-lived PSUM slot tenant
                tag_i[0] += 1
                wps = ps_pool.tile([128, 2048], F32, tag="ps",
                                   name=f"wps{tag_i[0]}")
                for i in range(n):
                    nc.tensor.matmul(wps[:, 0:512], lhsT=warm_sb[:, 0:128],
                                     rhs=warm_sb, start=True, stop=True)

            def emit_qproj_cols(lo, hi, name):
                # project q for query columns [lo, hi) of this core's chunk
                ps_q = ps_pool.tile([128, 2048], F32, tag="ps", name=name)
                for j in range((hi - lo) // 512):
                    osl = slice(j * 512, (j + 1) * 512)
                    src = bass.ds(qoff + lo + j * 512, 512)
                    nc.tensor.matmul(ps_q[:, osl], lhsT=w_sb[:, 0, 0:DIM_HEAD],
                                     rhs=fk_sb[:, 0, src],
                                     start=True, stop=False)
                    nc.tensor.matmul(ps_q[:, osl], lhsT=w_sb[:, 1, 0:DIM_HEAD],
                                     rhs=fk_sb[:, 1, src],
                                     start=False, stop=True)
                nc.vector.tensor_copy(q_sb[:, lo:hi], ps_q[:, 0:hi - lo])

            # The q projection (dynamic offset -> conservative dep on the
            # whole fk tile) and the last k chunk both unblock when the last
            # fmap chunk lands; everything before runs during the load.
            # Warm-keeper matmuls fill PE idle between chunks so the
            # post-load chain runs at 2.4 GHz. Only the first 512 query
            # columns are projected before tile 0 (they cover tiles 0-3);
            # the rest happens in the shadow of tile 0's softmax.
            emit_kproj(0)
            emit_warm(6)
            emit_kproj(1)
            emit_warm(6)
            emit_qproj_cols(0, 512, "ps_qa")

            # ---- scores + softmax, 16 query tiles of 128 ----
            for qt in range(QCHUNK // 128):
                if qt == 1:
                    emit_qproj_cols(512, QCHUNK, "ps_qb")
                qsl = q_sb[:, qt * 128:(qt + 1) * 128]
                et = soft_pool.tile([128, XY], F32, tag="et")
                # Tile 0 splits the exp into 1024-wide chunks so the first
                # store only waits on the last k chunk's 512-wide matmuls,
                # not a whole 2048-wide exp. Steady-state tiles use the
                # cheaper 2-instruction exp.
                nexp = 4 if qt == 0 else 2
                ech = XY // nexp
                pp = small_pool.tile([128, 4], F32, tag="pp")
                for half in range(2):
                    ps = ps_pool.tile([128, 2048], F32, tag="ps")
                    for j in range(4):
                        osl = slice(j * 512, (j + 1) * 512)
                        ksl = slice(half * 2048 + j * 512, half * 2048 + (j + 1) * 512)
                        nc.tensor.matmul(ps[:, osl], lhsT=qsl,
                                         rhs=k_sb[:, ksl],
                                         start=True, stop=True)
                    # exp straight out of PSUM, with per-row partial sums
                    # accumulated for free.
                    for e in range(nexp // 2):
                        psl = slice(e * ech, (e + 1) * ech)
                        idx = half * (nexp // 2) + e
                        nc.scalar.activation(
                            out=et[:, half * 2048 + e * ech:
                                   half * 2048 + (e + 1) * ech],
                            in_=ps[:, psl],
                            func=mybir.ActivationFunctionType.Exp,
                            accum_out=pp[:, idx:idx + 1])
                den = small_pool.tile([128, 1], F32, tag="den")
                if nexp == 2:
                    nc.vector.tensor_add(den, pp[:, 0:1], pp[:, 1:2])
                else:
                    nc.vector.tensor_reduce(den, pp[:, 0:nexp],
                                            axis=mybir.AxisListType.X,
                                            op=mybir.AluOpType.add)
                nc.vector.reciprocal(den, den)
                if qt == 0:
                    # normalize + store in halves: the first bytes hit HBM
                    # ~1.2us sooner, shrinking the post-load DMA hole
                    for h2 in range(2):
                        sl2 = slice(h2 * 2048, (h2 + 1) * 2048)
                        nc.vector.tensor_scalar_mul(et[:, sl2], et[:, sl2], den)
                        nc.sync.dma_start(
                            out=out[qt * 128:(qt + 1) * 128, sl2],
                            in_=et[:, sl2])
                else:
                    nc.vector.tensor_scalar_mul(et, et, den)
                    nc.sync.dma_start(out=out[qt * 128:(qt + 1) * 128, :],
                                      in_=et)


def build_program():
    nc = bacc.Bacc("TRN2", target_bir_lowering=False, debug=False,
                   enable_asserts=False)
    fmap_k = nc.dram_tensor("fmap_k", [C, XY], F32R, kind="ExternalInput").ap()
    wqkt = nc.dram_tensor("wqkt", [C, 2 * DIM_HEAD], F32R,
                          kind="ExternalInput").ap()
    out = nc.dram_tensor("out", [QCHUNK, XY], F32, kind="ExternalOutput").ap()

    with tile.TileContext(nc) as tc:
        _emit(tc, fmap_k, wqkt, out)
    nc.compile()
    return nc


_CACHE = {}


def _get_nc():
    if "nc" not in _CACHE:
        _CACHE["nc"] = build_program()
    return _CACHE["nc"]


def make_in_maps(fmap, W_qk):
    fm = np.ascontiguousarray(np.asarray(fmap, dtype=np.float32).reshape(C, XY))
    W = np.asarray(W_qk, dtype=np.float32)
    in_maps = []
    for core in range(N_CORES):
        hd, qhalf = divmod(core, 2)
        wq = W[hd * DIM_HEAD:(hd + 1) * DIM_HEAD] * np.float32(SCALE)
        wk = W[HEADS * DIM_HEAD + hd * DIM_HEAD:
               HEADS * DIM_HEAD + (hd + 1) * DIM_HEAD]
        in_maps.append({
            "fmap_k": fm,
            "wqkt": np.ascontiguousarray(np.concatenate([wq.T, wk.T], axis=1)),
        })
    return in_maps


def assemble(per_core_outs):
    out = np.empty((HEADS, XY, XY), dtype=np.float32)
    for core in range(N_CORES):
        hd, qhalf = divmod(core, 2)
        out[hd, qhalf * QCHUNK:(qhalf + 1) * QCHUNK, :] = per_core_outs[core]
    return out.reshape(1, HEADS, XY, XY)


def kernel(fmap, W_qk, trace=False):
    nc = _get_nc()
    in_maps = make_in_maps(fmap, W_qk)
    res = bass_utils.run_bass_kernel_spmd(
        nc, in_maps, core_ids=list(range(N_CORES)), trace=trace)
    out = assemble([res.results[c]["out"] for c in range(N_CORES)])
    if trace:
        kernel.last_exec_time_ns = res.exec_time_ns
        kernel.last_results = res
    return out
